# revision 6
# baseline (speedup 1.0000x reference)
"""Majority-vote (binary bincount+argmax) Trainium2 Bass kernel.

Problem: inputs [31, 2_000_000] int32 with values in {0, 1}. For each batch
column, output argmax of the class histogram = 1 iff sum of the 31 votes
>= 16 (31 is odd, so no ties), else 0. Output: [2_000_000] int32.

Strategy: pure data-parallel across 8 NeuronCores — each core gets a
contiguous 250_000-column slice, zero-padded to 256_000 = 125 x 2048.
The host transposes each core's slice to [125, 31, 2048] (partition-major)
so all 31 voter lines of a partition are contiguous in DRAM: loads become
8 big multi-voter DMAs with long contiguous runs on both sides instead of
31 per-voter DMAs (SWDGE Q7 descriptor generation was pacing the
pipeline). Loads are issued on nc.gpsimd with int32->int16 cast in the
SDMA datapath: HBM reads unchanged, SBUF-port writes halved (the binding
side with all 8 cores loaded; 2:1 SDMA->port mux), DVE int16 adds run 2x
(2x_1P mode). Serial DVE accumulate, tensor_scalar is_ge(16), cast store.
Memory-bound: ~31.75 MB read/core (~89 us HBM floor).
"""

import numpy as np

V = 31                  # voters
BATCH = 2_000_000
N_CORES = 8
B = BATCH // N_CORES    # 250_000 batch columns per core
P = 125                 # SBUF partitions used
Q = 2048                # free elems/partition; 125*2048 = 256_000 (padded)
BP = P * Q              # padded per-core batch
GK = 4                  # voters per load DMA (groups: 4,4,4,4,4,4,4,3)
THRESH = (V + 1) // 2   # 16

_cache = {}


def _build_nc():
    import concourse.bacc as bacc
    import concourse.mybir as mybir
    from concourse.mybir import AluOpType
    from concourse.tile import TileContext

    nc = bacc.Bacc("TRN2", target_bir_lowering=False, debug=False)
    x = nc.dram_tensor("x", [P, V, Q], mybir.dt.int32, kind="ExternalInput")
    y = nc.dram_tensor("y", [P, Q], mybir.dt.int32, kind="ExternalOutput")

    groups = []
    v0 = 0
    while v0 < V:
        groups.append((v0, min(GK, V - v0)))
        v0 += GK

    with TileContext(nc) as tc:
        with tc.tile_pool(name="vt", bufs=len(groups)) as vpool, \
             tc.tile_pool(name="acc", bufs=1) as apool, \
             tc.tile_pool(name="ot", bufs=1) as opool:
            acc = apool.tile([P, Q], mybir.dt.int16)
            tiles = []
            for v0, k in groups:
                t = vpool.tile([P, k * Q], mybir.dt.int16)
                nc.gpsimd.dma_start(t[:], x[:, v0:v0 + k, :])
                tiles.append((t, k))
            first = True
            prev = None
            for t, k in tiles:
                for j in range(k):
                    sl = t[:, j * Q:(j + 1) * Q]
                    if first:
                        prev = sl
                        first = False
                    elif prev is not None:
                        nc.vector.tensor_tensor(acc[:], prev, sl, AluOpType.add)
                        prev = None
                    else:
                        nc.vector.tensor_tensor(acc[:], acc[:], sl, AluOpType.add)
            ot = opool.tile([P, Q], mybir.dt.int16)
            nc.vector.tensor_scalar(ot[:], acc[:], THRESH, None, AluOpType.is_ge)
            # cast back int16 -> int32 on the store (SWDGE again)
            nc.gpsimd.dma_start(y[:, :], ot[:])
    nc.compile()
    return nc


def _get_nc():
    if "nc" not in _cache:
        _cache["nc"] = _build_nc()
    return _cache["nc"]


def _run(in_maps, **kwargs):
    from concourse.bass_utils import run_bass_kernel_spmd

    return run_bass_kernel_spmd(
        _get_nc(), in_maps, core_ids=list(range(N_CORES)), **kwargs
    )


def _shard(inputs):
    in_maps = []
    for i in range(N_CORES):
        xi = np.zeros((V, BP), dtype=np.int32)
        xi[:, :B] = inputs[:, i * B:(i + 1) * B]
        # [V, P, Q] -> [P, V, Q]: per partition, voter lines contiguous
        xt = np.ascontiguousarray(
            xi.reshape(V, P, Q).transpose(1, 0, 2))
        in_maps.append({"x": xt})
    return in_maps


def _gather(results):
    out = np.empty(BATCH, dtype=np.int32)
    for i in range(N_CORES):
        out[i * B:(i + 1) * B] = results[i]["y"].reshape(BP)[:B]
    return out


def kernel(inputs: np.ndarray) -> np.ndarray:
    inputs = np.asarray(inputs)
    assert inputs.shape == (V, BATCH), inputs.shape
    inputs = inputs.astype(np.int32, copy=False)
    res = _run(_shard(inputs))
    return _gather(res.results)
